# revision 23
# baseline (speedup 1.0000x reference)
"""Trainium2 Bass kernel for nn_CustomMultiHeadAttention (B2 T2048 D1024 H16).

Sharding: 8 cores = 2 batches x 4 head-groups (4 heads/core, tensor-parallel
columns for Wq/Wk/Wv, rows for Wo; host sums the 4 row-parallel partials).

Math: F_ij = bs*(fj-fi)/(fi*fj+eps) ~= bs*(1/fi - 1/fj).  The 1/fi row term
is softmax-invariant, so F collapses to a per-column logit bias
c_j = -bs*scale/f_j.

Layout trick: compute S TRANSPOSED (S^T[j,i] = K_j . Q_i).  Then:
  * c_j is per-PARTITION -> folds into the ACT exp bias (exact fp32, free,
    together with a fixed safe shift A=30 >= max |scale*QK| by
    Cauchy-Schwarz on randn inputs);
  * exp output P~^T is already in the [j, i] layout the PV matmul needs --
    zero DMA transposes of the 32MB probability matrix;
  * softmax denominators come free as a 65th all-ones V column in PV
    (row 64 of each head's O^T accumulator);
  * normalization: small per-head O^T -> O transpose (PE), divide by the
    denom column (DVE, per-partition), transpose back -> paired out-proj.

Pipeline: proj Q,K -> S^T/exp for the first head-pair starts on ACT while
the V projection + V transpose run on PE (separate PSUM pools); PV trails
the exp stream using per-jc probability tiles (deep slot pool, no WAR
stalls); normalization dance + out-proj interleaved at the tail.
"""

from contextlib import ExitStack

import numpy as np
import ml_dtypes

import concourse.bass as bass
import concourse.mybir as mybir
import concourse.tile as tile
from concourse import bacc
from concourse.bass_utils import run_bass_kernel_spmd
from concourse.masks import make_identity

AF = mybir.ActivationFunctionType
ALU = mybir.AluOpType
F32 = mybir.dt.float32
BF16 = mybir.dt.bfloat16
X = mybir.AxisListType.X

B, T, D = 2, 2048, 1024
H, DH = 16, 64
H_LOC = 4
C_LOC = H_LOC * DH          # 256
N_CORES = 8
SCALE = DH ** -0.5
P = 128
ICH, JCH, KCH = T // P, T // P, D // P   # 16, 16, 8
SL2 = 2                     # i-slices
IW = T // SL2               # 1024 i-columns per slice
VW = DH + 1                 # 65: V columns + ones (denominator)
A_SHIFT = 30.0              # fixed exp upper bound on |scale*QK|


def _build_program():
    nc = bacc.Bacc("TRN2", target_bir_lowering=False, debug=False,
                   num_devices=N_CORES)

    # x tensors host-pre-tiled [kc*2+th, 128, 1024] and weights [p, kc, c]
    # so every DMA lands in few large contiguous descriptors
    xq_d = nc.dram_tensor("xq", [2 * KCH, P, 1024], BF16,
                          kind="ExternalInput").ap()
    xk_d = nc.dram_tensor("xk", [2 * KCH, P, 1024], BF16,
                          kind="ExternalInput").ap()
    xv_d = nc.dram_tensor("xv", [2 * KCH, P, 1024], BF16,
                          kind="ExternalInput").ap()
    wq_d = nc.dram_tensor("wq", [P, KCH, C_LOC], BF16,
                          kind="ExternalInput").ap()
    wk_d = nc.dram_tensor("wk", [P, KCH, C_LOC], BF16,
                          kind="ExternalInput").ap()
    wv_d = nc.dram_tensor("wv", [P, KCH, C_LOC], BF16,
                          kind="ExternalInput").ap()
    wo_d = nc.dram_tensor("wo", [P, 2, D], BF16, kind="ExternalInput").ap()
    cb_d = nc.dram_tensor("cb", [P, JCH], F32, kind="ExternalInput").ap()
    out_d = nc.dram_tensor("out", [T, D], BF16, kind="ExternalOutput").ap()

    with tile.TileContext(nc) as tc, ExitStack() as ctx:
        const = ctx.enter_context(tc.tile_pool(name="const", bufs=1))
        wpool = ctx.enter_context(tc.tile_pool(name="w", bufs=1))
        qkv = ctx.enter_context(tc.tile_pool(name="qkv", bufs=1))
        xpool = ctx.enter_context(tc.tile_pool(name="x", bufs=8))
        rvpool = ctx.enter_context(tc.tile_pool(name="rv", bufs=8))
        ptpool = ctx.enter_context(tc.tile_pool(name="pt", bufs=20))
        onpool = ctx.enter_context(tc.tile_pool(name="on", bufs=4))
        opool = ctx.enter_context(tc.tile_pool(name="o", bufs=3))
        psum = ctx.enter_context(tc.tile_pool(name="ps", bufs=2, space="PSUM"))
        pvps = ctx.enter_context(tc.tile_pool(name="pv", bufs=2, space="PSUM"))

        identb = const.tile([P, P], BF16)
        make_identity(nc, identb)
        cb_s = const.tile([P, JCH], F32)
        nc.gpsimd.dma_start(cb_s[:], cb_d)

        # weight DMAs ordered by need: wq ahead of the xk stream (scalar
        # queue); wk/wv/wo behind cb on gpsimd, ahead of the late xv stream.
        wq_s = wpool.tile([P, KCH, C_LOC], BF16, tag="wq")
        nc.scalar.dma_start(wq_s[:], wq_d)
        wk_s = wpool.tile([P, KCH, C_LOC], BF16, tag="wk")
        nc.gpsimd.dma_start(wk_s[:], wk_d)
        # wv/wo DMAs are deferred to just before their consumers so the
        # gpsimd queue stays clear during the x-hungry warmup
        wv_s = wpool.tile([P, KCH, C_LOC], BF16, tag="wv")
        wo_s = wpool.tile([P, 2, D], BF16, tag="wo")

        # ---- projections: dst[c % 128, pair, t] = (W.T x^T)  bf16 ----
        # x loads spread across the three DGE paths (sync/scalar/gpsimd)
        # so the streams drain in parallel.
        qt_s = qkv.tile([P, 2, T], BF16, tag="qt")
        kt_s = qkv.tile([P, 2, T], BF16, tag="kt")
        # vt (projection staging) and ot65 (PV output) have disjoint
        # lifetimes -> share one slot.  bf16 staging: V transposes run in
        # 1-cycle/row mode instead of fp32's 2.
        vt_s = qkv.tile([P, 2, T], BF16, tag="big")

        def _proj_th(x_d, w_s, dst, dma_engs, xtag, ppool, copy_fn, th):
            t0 = th * 1024
            pstiles = [ppool.tile([P, 1024], F32,
                                  tag="ps" if ppool is psum else "pv",
                                  name=f"pj{xtag}{th}{pi}")
                       for pi in range(2)]
            for kc in range(KCH):
                xt = xpool.tile([P, 1024], BF16, tag=xtag)
                dma_engs[kc % len(dma_engs)].dma_start(
                    xt[:], x_d[kc * 2 + th])
                for pair in range(2):
                    lhsT = w_s[:, kc, pair * P:(pair + 1) * P]
                    for nb in range(2):
                        nc.tensor.matmul(
                            pstiles[pair][:, nb * 512:(nb + 1) * 512],
                            lhsT, xt[:, nb * 512:(nb + 1) * 512],
                            start=(kc == 0), stop=(kc == KCH - 1))
            for pair in range(2):
                copy_fn(dst[:, pair, t0:t0 + 1024], pstiles[pair][:])

        def _proj(x_d, w_s, dst, dma_engs, xtag, ppool, copy_fn):
            for th in range(2):                      # halves of T
                _proj_th(x_d, w_s, dst, dma_engs, xtag, ppool, copy_fn, th)

        # ---- V^T -> V[t % 128, tc, h*65 + c] bf16 via PE transpose ----
        # 65-column head stride; column h*65+64 stays 1.0 (denominator);
        # only those 4 columns need the memset (the rest is overwritten).
        v_s = qkv.tile([P, ICH, H_LOC * VW], BF16, tag="v")
        for h in range(H_LOC):
            nc.any.memset(v_s[:, :, h * VW + DH:h * VW + VW], 1.0)

        def _v_transpose():
            for tc_i in range(ICH):
                for pair in range(2):
                    tp = pvps.tile([P, IW], BF16, tag="pv",
                                   name=f"tp{tc_i}_{pair}")
                    nc.tensor.transpose(
                        tp[:, 0:P], vt_s[:, pair, tc_i * P:(tc_i + 1) * P],
                        identb[:])
                    for hh in range(2):
                        h = 2 * pair + hh
                        nc.vector.tensor_copy(
                            v_s[:, tc_i, h * VW:h * VW + DH],
                            tp[:, hh * DH:(hh + 1) * DH])

        otn = qkv.tile([P, 2, T], BF16, tag="otn")
        # bf16 staging: halves the fp32 transpose-mode penalty (2 cyc/row)
        # and the flush copies; num/denom round independently (~0.3% out)
        ot65 = qkv.tile([VW, SL2 * H_LOC, IW], BF16, tag="big",
                        name="ot65")

        # ---- main loop pieces ----
        pt_tiles = {}

        def _sexp_jc(sl2, pair, jc):
            """S^T matmuls + exp for one (i-slice, head-pair, j-chunk)."""
            i0 = sl2 * IW
            sp = [psum.tile([P, IW], F32, tag="ps",
                            name=f"sp{sl2}{pair}{jc}{hh}")
                  for hh in range(2)]
            # ib-outer: consecutive matmuls alternate PE row-halves
            # (tile_position), letting each weight load hide under the
            # other half's stream
            for ib in range(2):
                for hh in range(2):
                    nc.tensor.matmul(
                        sp[hh][:, ib * 512:(ib + 1) * 512],
                        kt_s[hh * 64:(hh + 1) * 64, pair,
                             jc * P:(jc + 1) * P],
                        qt_s[hh * 64:(hh + 1) * 64, pair,
                             i0 + ib * 512:i0 + (ib + 1) * 512],
                        start=True, stop=True,
                        tile_position=(64 * hh, 0))
            for hh in range(2):
                ptt = ptpool.tile([P, IW], BF16, tag=f"pt{hh}",
                                  name=f"pt{hh}_{sl2}{pair}{jc}")
                pt_tiles[(sl2, pair, jc, hh)] = ptt
                nc.scalar.activation(
                    ptt[:], sp[hh][:],
                    AF.Exp, bias=cb_s[:, jc:jc + 1], scale=SCALE)

        def _pv_jc(sl2, pair, jc, ov):
            """PV chunk with 65th ones-column (denominator in row 64)."""
            for hh in range(2):
                h = 2 * pair + hh
                ptt = pt_tiles[(sl2, pair, jc, hh)]
                for ib in range(2):
                    nc.tensor.matmul(
                        ov[hh][:, ib * 512:(ib + 1) * 512],
                        v_s[:, jc, h * VW:(h + 1) * VW],
                        ptt[:, ib * 512:(ib + 1) * 512],
                        start=(jc == 0), stop=(jc == JCH - 1))

        def _ov_flush(sl2, pair, ov):
            for hh in range(2):
                k = sl2 * H_LOC + 2 * pair + hh
                nc.vector.tensor_copy(ot65[:, k, :], ov[hh][:])

        def _ov_alloc(sl2, pair, pool):
            return [pool.tile([VW, IW], F32,
                              tag="pv" if pool is pvps else "ps",
                              name=f"ov{sl2}{pair}{hh}") for hh in range(2)]

        # ---- normalize + repack: O^T[65,i] -> O -> /denom -> O^T paired,
        # split in two stages so unrelated PE work can cover the DVE
        # recip/mul latency between them ----
        def _dance_a(sl2, pair, ib):
            tp = pvps.tile([P, IW], BF16, tag="pv",
                           name=f"dtp{sl2}{pair}{ib}")
            onorm = onpool.tile([P, P], BF16, tag="on",
                                name=f"on{sl2}{pair}{ib}")
            for hh in range(2):
                k = sl2 * H_LOC + 2 * pair + hh
                nc.tensor.transpose(
                    tp[:, hh * 66:hh * 66 + VW],
                    ot65[:, k, ib * P:(ib + 1) * P],
                    identb[0:VW, 0:VW])
            for hh in range(2):
                rv = rvpool.tile([P, 1], F32, tag="rv",
                                 name=f"rv{sl2}{pair}{ib}{hh}")
                nc.vector.reciprocal(
                    rv[:], tp[:, hh * 66 + DH:hh * 66 + VW])
                nc.vector.tensor_scalar_mul(
                    onorm[:, hh * DH:(hh + 1) * DH],
                    tp[:, hh * 66:hh * 66 + DH], rv[:])
            return onorm

        def _dance_b(sl2, pair, ib, onorm):
            tpo = pvps.tile([P, 1024], BF16, tag="pv",
                            name=f"dto{sl2}{pair}{ib}")
            nc.tensor.transpose(tpo[:, 0:P], onorm[:], identb[:])
            nc.scalar.copy(
                otn[:, pair, sl2 * IW + ib * P:sl2 * IW + (ib + 1) * P],
                tpo[:, 0:P])

        def _dance(sl2, pair, ib):
            _dance_b(sl2, pair, ib, _dance_a(sl2, pair, ib))

        def _outproj(tb):
            ops = psum.tile([P, 1024], F32, tag="ps", name=f"op{tb}")
            for cc in range(2):
                lhsT = otn[:, cc, tb * P:(tb + 1) * P]
                for nb in range(2):
                    nc.tensor.matmul(
                        ops[:, nb * 512:(nb + 1) * 512], lhsT,
                        wo_s[:, cc, nb * 512:(nb + 1) * 512],
                        start=(cc == 0), stop=(cc == 1))
            ostage = opool.tile([P, D], BF16, tag="ostage")
            # mostly ACT with some DVE: DVE already carries the dance work
            if tb % 4 == 3:
                nc.vector.tensor_copy(ostage[:], ops[:])
            else:
                nc.scalar.copy(ostage[:], ops[:])
            nc.sync.dma_start(out_d[tb * P:(tb + 1) * P, :], ostage[:])

        # ---- emission: proj Q,K -> first S/exp overlaps proj V; each
        # later pair's exp stream interleaves the previous pair's PV at
        # j-chunk granularity (constant pt-slot pressure, no ACT stalls)
        _proj(xq_d, wq_s, qt_s, (nc.sync, nc.scalar), "xq", psum,
              nc.scalar.copy)
        _proj(xk_d, wk_s, kt_s, (nc.scalar, nc.sync), "xk", psum,
              nc.scalar.copy)
        nc.gpsimd.dma_start(wv_s[:], wv_d)
        nc.gpsimd.dma_start(wo_s[:], wo_d)
        for jc in range(JCH):
            _sexp_jc(0, 0, jc)
        _proj(xv_d, wv_s, vt_s, (nc.gpsimd, nc.sync), "xv", pvps,
              nc.vector.tensor_copy)
        _v_transpose()
        steps = [(0, 1), (1, 0), (1, 1)]
        prev = (0, 0)
        ov = _ov_alloc(*prev, pvps)
        for cur in steps:
            for jc in range(JCH):
                _sexp_jc(cur[0], cur[1], jc)
                _pv_jc(prev[0], prev[1], jc, ov)
            _ov_flush(*prev, ov)
            prev = cur
            # the last pair's PV accumulates in the ps pool (sp tiles are
            # dead by then) so the dance can claim pv slots while it drains
            ov = _ov_alloc(*prev, psum if cur == (1, 1) else pvps)
        # final PV drain with the sl2=0 dances woven in: PE has slack
        # (PV-only j-chunks) and DVE/ACT are otherwise idle here
        dance_q = [(0, pair, ib) for ib in range(IW // P) for pair in range(2)]
        for jc in range(JCH):
            _pv_jc(prev[0], prev[1], jc, ov)
            _dance(*dance_q[jc])
        _ov_flush(*prev, ov)
        # sl2=1 dances pipelined against the sl2=0 out-projections: the
        # outproj matmuls sit between dance stage A (tp transposes + DVE
        # recip/mul) and stage B (transpose back) so PE never waits on DVE
        for ib in range(IW // P):
            on0 = _dance_a(1, 0, ib)
            on1 = _dance_a(1, 1, ib)
            _outproj(ib)
            _dance_b(1, 0, ib, on0)
            _dance_b(1, 1, ib, on1)
        for ib in range(IW // P):
            _outproj((IW // P) + ib)

    nc.compile()
    return nc


_last_results = None


def _host_cb(frac: np.ndarray, bs: float):
    """Per-j exp bias cb[p, jc] = scale*(c_j - maxc) - A, j = jc*128 + p,
    with c_j = -bs/f_j (raw logit units)."""
    cbs = []
    for b in range(B):
        f = np.maximum(frac[b].astype(np.float64), 1e-7)
        c = -bs / f
        cb = SCALE * (c - c.max()) - A_SHIFT
        cbs.append(np.ascontiguousarray(
            cb.reshape(JCH, P).T.astype(np.float32)))
    return cbs


def _prepare(inputs):
    """Build the program and per-core input maps from full inputs."""
    inp = {k: np.asarray(v) for k, v in inputs.items()}
    query, key, value = inp["query"], inp["key"], inp["value"]
    frac = inp["frac"]
    Wq, Wk, Wv, Wo = inp["Wq"], inp["Wk"], inp["Wv"], inp["Wo"]
    attn_bias = inp["attn_bias"]

    bs = float(np.sum(attn_bias.astype(np.float64)))
    cbs = _host_cb(np.asarray(frac, np.float32), bs)

    nc = _build_program()

    def _tile_x(x):
        # [T, D] -> x^T [D, T] -> [kc*2+th, 128, 1024] contiguous chunks
        xt = x.T.reshape(KCH, P, 2, 1024).transpose(0, 2, 1, 3)
        return np.ascontiguousarray(xt.reshape(2 * KCH, P, 1024)
                                    ).astype(ml_dtypes.bfloat16)

    def _tile_w(w):
        # W[sl] [256, D] -> W.T [D, 256] -> [p, kc, 256]
        wt = w.T.reshape(KCH, P, C_LOC).transpose(1, 0, 2)
        return np.ascontiguousarray(wt).astype(ml_dtypes.bfloat16)

    in_maps = []
    for c in range(N_CORES):
        b, g = c // H_LOC, c % H_LOC
        sl = slice(g * C_LOC, (g + 1) * C_LOC)
        wo_t = Wo[:, sl].T.reshape(2, P, D).transpose(1, 0, 2)
        in_maps.append({
            "xq": _tile_x(query[b]),
            "xk": _tile_x(key[b]),
            "xv": _tile_x(value[b]),
            "wq": _tile_w(Wq[sl, :]),
            "wk": _tile_w(Wk[sl, :]),
            "wv": _tile_w(Wv[sl, :]),
            "wo": np.ascontiguousarray(wo_t).astype(ml_dtypes.bfloat16),
            "cb": cbs[b],
        })
    return nc, in_maps


def kernel(**inputs) -> np.ndarray:
    nc, in_maps = _prepare(inputs)

    res = run_bass_kernel_spmd(nc, in_maps, list(range(N_CORES)))
    global _last_results
    _last_results = res

    out = np.zeros((B, T, D), dtype=np.float32)
    for c in range(N_CORES):
        out[c // H_LOC] += np.asarray(res.results[c]["out"]).astype(np.float32)
    out += np.asarray(inputs["bo"], dtype=np.float32)[None, None, :]
    return out



# revision 25
# speedup vs baseline: 1.0122x; 1.0122x over previous
"""Trainium2 Bass kernel for nn_CustomMultiHeadAttention (B2 T2048 D1024 H16).

Sharding: 8 cores = 2 batches x 4 head-groups (4 heads/core, tensor-parallel
columns for Wq/Wk/Wv, rows for Wo; host sums the 4 row-parallel partials).

Math: F_ij = bs*(fj-fi)/(fi*fj+eps) ~= bs*(1/fi - 1/fj).  The 1/fi row term
is softmax-invariant, so F collapses to a per-column logit bias
c_j = -bs*scale/f_j.

Layout trick: compute S TRANSPOSED (S^T[j,i] = K_j . Q_i).  Then:
  * c_j is per-PARTITION -> folds into the ACT exp bias (exact fp32, free,
    together with a fixed safe shift A=30 >= max |scale*QK| by
    Cauchy-Schwarz on randn inputs);
  * exp output P~^T is already in the [j, i] layout the PV matmul needs --
    zero DMA transposes of the 32MB probability matrix;
  * softmax denominators come free as a 65th all-ones V column in PV
    (row 64 of each head's O^T accumulator);
  * normalization: small per-head O^T -> O transpose (PE), divide by the
    denom column (DVE, per-partition), transpose back -> paired out-proj.

Pipeline: proj Q,K -> S^T/exp for the first head-pair starts on ACT while
the V projection + V transpose run on PE (separate PSUM pools); PV trails
the exp stream using per-jc probability tiles (deep slot pool, no WAR
stalls); normalization dance + out-proj interleaved at the tail.
"""

from contextlib import ExitStack

import numpy as np
import ml_dtypes

import concourse.bass as bass
import concourse.mybir as mybir
import concourse.tile as tile
from concourse import bacc
from concourse.bass_utils import run_bass_kernel_spmd
from concourse.masks import make_identity

AF = mybir.ActivationFunctionType
ALU = mybir.AluOpType
F32 = mybir.dt.float32
BF16 = mybir.dt.bfloat16
X = mybir.AxisListType.X

B, T, D = 2, 2048, 1024
H, DH = 16, 64
H_LOC = 4
C_LOC = H_LOC * DH          # 256
N_CORES = 8
SCALE = DH ** -0.5
P = 128
ICH, JCH, KCH = T // P, T // P, D // P   # 16, 16, 8
SL2 = 2                     # i-slices
IW = T // SL2               # 1024 i-columns per slice
VW = DH + 1                 # 65: V columns + ones (denominator)
A_SHIFT = 30.0              # fixed exp upper bound on |scale*QK|


def _build_program():
    nc = bacc.Bacc("TRN2", target_bir_lowering=False, debug=False,
                   num_devices=N_CORES)

    # x tensors host-pre-tiled [kc*2+th, 128, 1024] and weights [p, kc, c]
    # so every DMA lands in few large contiguous descriptors
    xq_d = nc.dram_tensor("xq", [2 * KCH, P, 1024], BF16,
                          kind="ExternalInput").ap()
    xk_d = nc.dram_tensor("xk", [2 * KCH, P, 1024], BF16,
                          kind="ExternalInput").ap()
    xv_d = nc.dram_tensor("xv", [2 * KCH, P, 1024], BF16,
                          kind="ExternalInput").ap()
    wq_d = nc.dram_tensor("wq", [P, KCH, C_LOC], BF16,
                          kind="ExternalInput").ap()
    wk_d = nc.dram_tensor("wk", [P, KCH, C_LOC], BF16,
                          kind="ExternalInput").ap()
    wv_d = nc.dram_tensor("wv", [P, KCH, C_LOC], BF16,
                          kind="ExternalInput").ap()
    wo_d = nc.dram_tensor("wo", [P, 2, D], BF16, kind="ExternalInput").ap()
    cb_d = nc.dram_tensor("cb", [P, JCH], F32, kind="ExternalInput").ap()
    out_d = nc.dram_tensor("out", [T, D], BF16, kind="ExternalOutput").ap()

    with tile.TileContext(nc) as tc, ExitStack() as ctx:
        const = ctx.enter_context(tc.tile_pool(name="const", bufs=1))
        wpool = ctx.enter_context(tc.tile_pool(name="w", bufs=1))
        qkv = ctx.enter_context(tc.tile_pool(name="qkv", bufs=1))
        xpool = ctx.enter_context(tc.tile_pool(name="x", bufs=8))
        rvpool = ctx.enter_context(tc.tile_pool(name="rv", bufs=8))
        ptpool = ctx.enter_context(tc.tile_pool(name="pt", bufs=20))
        onpool = ctx.enter_context(tc.tile_pool(name="on", bufs=4))
        opool = ctx.enter_context(tc.tile_pool(name="o", bufs=3))
        psum = ctx.enter_context(tc.tile_pool(name="ps", bufs=2, space="PSUM"))
        pvps = ctx.enter_context(tc.tile_pool(name="pv", bufs=2, space="PSUM"))

        identb = const.tile([P, P], BF16)
        make_identity(nc, identb)
        cb_s = const.tile([P, JCH], F32)
        nc.gpsimd.dma_start(cb_s[:], cb_d)

        # weight DMAs ordered by need: wq ahead of the xk stream (scalar
        # queue); wk/wv/wo behind cb on gpsimd, ahead of the late xv stream.
        wq_s = wpool.tile([P, KCH, C_LOC], BF16, tag="wq")
        nc.scalar.dma_start(wq_s[:], wq_d)
        wk_s = wpool.tile([P, KCH, C_LOC], BF16, tag="wk")
        nc.gpsimd.dma_start(wk_s[:], wk_d)
        # wv/wo DMAs are deferred to just before their consumers so the
        # gpsimd queue stays clear during the x-hungry warmup
        wv_s = wpool.tile([P, KCH, C_LOC], BF16, tag="wv")
        wo_s = wpool.tile([P, 2, D], BF16, tag="wo")

        # ---- projections: dst[c % 128, pair, t] = (W.T x^T)  bf16 ----
        # x loads spread across the three DGE paths (sync/scalar/gpsimd)
        # so the streams drain in parallel.
        qt_s = qkv.tile([P, 2, T], BF16, tag="qt")
        kt_s = qkv.tile([P, 2, T], BF16, tag="kt")
        # vt (projection staging) and ot65 (PV output) have disjoint
        # lifetimes -> share one slot.  bf16 staging: V transposes run in
        # 1-cycle/row mode instead of fp32's 2.
        vt_s = qkv.tile([P, 2, T], BF16, tag="big")

        def _proj_th(x_d, w_s, dst, dma_engs, xtag, ppool, copy_fn, th):
            t0 = th * 1024
            pstiles = [ppool.tile([P, 1024], F32,
                                  tag="ps" if ppool is psum else "pv",
                                  name=f"pj{xtag}{th}{pi}")
                       for pi in range(2)]
            for kc in range(KCH):
                xt = xpool.tile([P, 1024], BF16, tag=xtag)
                dma_engs[kc % len(dma_engs)].dma_start(
                    xt[:], x_d[kc * 2 + th])
                for pair in range(2):
                    lhsT = w_s[:, kc, pair * P:(pair + 1) * P]
                    for nb in range(2):
                        nc.tensor.matmul(
                            pstiles[pair][:, nb * 512:(nb + 1) * 512],
                            lhsT, xt[:, nb * 512:(nb + 1) * 512],
                            start=(kc == 0), stop=(kc == KCH - 1))
            for pair in range(2):
                copy_fn(dst[:, pair, t0:t0 + 1024], pstiles[pair][:])

        def _proj(x_d, w_s, dst, dma_engs, xtag, ppool, copy_fn):
            for th in range(2):                      # halves of T
                _proj_th(x_d, w_s, dst, dma_engs, xtag, ppool, copy_fn, th)

        # ---- V^T -> V[t % 128, tc, h*65 + c] bf16 via PE transpose ----
        # 65-column head stride; column h*65+64 stays 1.0 (denominator);
        # only those 4 columns need the memset (the rest is overwritten).
        v_s = qkv.tile([P, ICH, H_LOC * VW], BF16, tag="v")
        for h in range(H_LOC):
            nc.any.memset(v_s[:, :, h * VW + DH:h * VW + VW], 1.0)

        def _v_transpose():
            for tc_i in range(ICH):
                for pair in range(2):
                    tp = pvps.tile([P, IW], BF16, tag="pv",
                                   name=f"tp{tc_i}_{pair}")
                    nc.tensor.transpose(
                        tp[:, 0:P], vt_s[:, pair, tc_i * P:(tc_i + 1) * P],
                        identb[:])
                    for hh in range(2):
                        h = 2 * pair + hh
                        nc.vector.tensor_copy(
                            v_s[:, tc_i, h * VW:h * VW + DH],
                            tp[:, hh * DH:(hh + 1) * DH])

        otn = qkv.tile([P, 2, T], BF16, tag="otn")
        # bf16 staging: halves the fp32 transpose-mode penalty (2 cyc/row)
        # and the flush copies; num/denom round independently (~0.3% out)
        ot65 = qkv.tile([VW, SL2 * H_LOC, IW], BF16, tag="big",
                        name="ot65")

        # ---- main loop pieces ----
        pt_tiles = {}

        def _sexp_jc(sl2, pair, jc):
            """S^T matmuls + exp for one (i-slice, head-pair, j-chunk)."""
            i0 = sl2 * IW
            sp = [psum.tile([P, IW], F32, tag="ps",
                            name=f"sp{sl2}{pair}{jc}{hh}")
                  for hh in range(2)]
            # ib-outer: consecutive matmuls alternate PE row-halves
            # (tile_position), letting each weight load hide under the
            # other half's stream
            for ib in range(2):
                for hh in range(2):
                    nc.tensor.matmul(
                        sp[hh][:, ib * 512:(ib + 1) * 512],
                        kt_s[hh * 64:(hh + 1) * 64, pair,
                             jc * P:(jc + 1) * P],
                        qt_s[hh * 64:(hh + 1) * 64, pair,
                             i0 + ib * 512:i0 + (ib + 1) * 512],
                        start=True, stop=True,
                        tile_position=(64 * hh, 0))
            for hh in range(2):
                ptt = ptpool.tile([P, IW], BF16, tag=f"pt{hh}",
                                  name=f"pt{hh}_{sl2}{pair}{jc}")
                pt_tiles[(sl2, pair, jc, hh)] = ptt
                nc.scalar.activation(
                    ptt[:], sp[hh][:],
                    AF.Exp, bias=cb_s[:, jc:jc + 1], scale=SCALE)

        def _pv_jc(sl2, pair, jc, ov):
            """PV chunk with 65th ones-column (denominator in row 64).
            On the last j-chunk each head's flush copy is emitted right
            after its final matmul so the DVE copies overlap the other
            head's accumulation instead of stalling the next pair."""
            for hh in range(2):
                h = 2 * pair + hh
                ptt = pt_tiles[(sl2, pair, jc, hh)]
                for ib in range(2):
                    nc.tensor.matmul(
                        ov[hh][:, ib * 512:(ib + 1) * 512],
                        v_s[:, jc, h * VW:(h + 1) * VW],
                        ptt[:, ib * 512:(ib + 1) * 512],
                        start=(jc == 0), stop=(jc == JCH - 1))
                if jc == JCH - 1:
                    k = sl2 * H_LOC + 2 * pair + hh
                    nc.vector.tensor_copy(ot65[:, k, :], ov[hh][:])

        def _ov_alloc(sl2, pair, pool):
            return [pool.tile([VW, IW], F32,
                              tag="pv" if pool is pvps else "ps",
                              name=f"ov{sl2}{pair}{hh}") for hh in range(2)]

        # ---- normalize + repack: O^T[65,i] -> O -> /denom -> O^T paired,
        # split in two stages so unrelated PE work can cover the DVE
        # recip/mul latency between them ----
        def _dance_a(sl2, pair, ib):
            tp = pvps.tile([P, IW], BF16, tag="pv",
                           name=f"dtp{sl2}{pair}{ib}")
            onorm = onpool.tile([P, P], BF16, tag="on",
                                name=f"on{sl2}{pair}{ib}")
            for hh in range(2):
                k = sl2 * H_LOC + 2 * pair + hh
                nc.tensor.transpose(
                    tp[:, hh * 66:hh * 66 + VW],
                    ot65[:, k, ib * P:(ib + 1) * P],
                    identb[0:VW, 0:VW])
            for hh in range(2):
                rv = rvpool.tile([P, 1], F32, tag="rv",
                                 name=f"rv{sl2}{pair}{ib}{hh}")
                nc.vector.reciprocal(
                    rv[:], tp[:, hh * 66 + DH:hh * 66 + VW])
                nc.vector.tensor_scalar_mul(
                    onorm[:, hh * DH:(hh + 1) * DH],
                    tp[:, hh * 66:hh * 66 + DH], rv[:])
            return onorm

        def _dance_b(sl2, pair, ib, onorm):
            tpo = pvps.tile([P, 1024], BF16, tag="pv",
                            name=f"dto{sl2}{pair}{ib}")
            nc.tensor.transpose(tpo[:, 0:P], onorm[:], identb[:])
            nc.scalar.copy(
                otn[:, pair, sl2 * IW + ib * P:sl2 * IW + (ib + 1) * P],
                tpo[:, 0:P])

        def _dance(sl2, pair, ib):
            _dance_b(sl2, pair, ib, _dance_a(sl2, pair, ib))

        def _outproj(tb):
            ops = psum.tile([P, 1024], F32, tag="ps", name=f"op{tb}")
            for cc in range(2):
                lhsT = otn[:, cc, tb * P:(tb + 1) * P]
                for nb in range(2):
                    nc.tensor.matmul(
                        ops[:, nb * 512:(nb + 1) * 512], lhsT,
                        wo_s[:, cc, nb * 512:(nb + 1) * 512],
                        start=(cc == 0), stop=(cc == 1))
            ostage = opool.tile([P, D], BF16, tag="ostage")
            # ACT: the DVE queue carries the dance chain; copies there
            # would delay tile releases and stall PE
            nc.scalar.copy(ostage[:], ops[:])
            nc.sync.dma_start(out_d[tb * P:(tb + 1) * P, :], ostage[:])

        # ---- emission: proj Q,K -> first S/exp overlaps proj V; each
        # later pair's exp stream interleaves the previous pair's PV at
        # j-chunk granularity (constant pt-slot pressure, no ACT stalls)
        _proj(xq_d, wq_s, qt_s, (nc.sync, nc.scalar), "xq", psum,
              nc.scalar.copy)
        _proj(xk_d, wk_s, kt_s, (nc.scalar, nc.sync), "xk", psum,
              nc.scalar.copy)
        nc.gpsimd.dma_start(wv_s[:], wv_d)
        nc.gpsimd.dma_start(wo_s[:], wo_d)
        for jc in range(JCH):
            _sexp_jc(0, 0, jc)
        _proj(xv_d, wv_s, vt_s, (nc.gpsimd, nc.sync), "xv", pvps,
              nc.vector.tensor_copy)
        _v_transpose()
        steps = [(0, 1), (1, 0), (1, 1)]
        prev = (0, 0)
        ov = _ov_alloc(*prev, pvps)
        for cur in steps:
            for jc in range(JCH):
                _sexp_jc(cur[0], cur[1], jc)
                _pv_jc(prev[0], prev[1], jc, ov)
            prev = cur
            # the last pair's PV accumulates in the ps pool (sp tiles are
            # dead by then) so the dance can claim pv slots while it drains
            ov = _ov_alloc(*prev, psum if cur == (1, 1) else pvps)
        # final PV drain with the sl2=0 dances woven in: PE has slack
        # (PV-only j-chunks) and DVE/ACT are otherwise idle here
        dance_q = [(0, pair, ib) for ib in range(IW // P) for pair in range(2)]
        for jc in range(JCH):
            _pv_jc(prev[0], prev[1], jc, ov)
            _dance(*dance_q[jc])
        # sl2=1 dances pipelined against the sl2=0 out-projections: the
        # outproj matmuls sit between dance stage A (tp transposes + DVE
        # recip/mul) and stage B (transpose back) so PE never waits on DVE
        for ib in range(IW // P):
            on0 = _dance_a(1, 0, ib)
            on1 = _dance_a(1, 1, ib)
            _outproj(ib)
            _dance_b(1, 0, ib, on0)
            _dance_b(1, 1, ib, on1)
        for ib in range(IW // P):
            _outproj((IW // P) + ib)

    nc.compile()
    return nc


_last_results = None


def _host_cb(frac: np.ndarray, bs: float):
    """Per-j exp bias cb[p, jc] = scale*(c_j - maxc) - A, j = jc*128 + p,
    with c_j = -bs/f_j (raw logit units)."""
    cbs = []
    for b in range(B):
        f = np.maximum(frac[b].astype(np.float64), 1e-7)
        c = -bs / f
        cb = SCALE * (c - c.max()) - A_SHIFT
        cbs.append(np.ascontiguousarray(
            cb.reshape(JCH, P).T.astype(np.float32)))
    return cbs


def _prepare(inputs):
    """Build the program and per-core input maps from full inputs."""
    inp = {k: np.asarray(v) for k, v in inputs.items()}
    query, key, value = inp["query"], inp["key"], inp["value"]
    frac = inp["frac"]
    Wq, Wk, Wv, Wo = inp["Wq"], inp["Wk"], inp["Wv"], inp["Wo"]
    attn_bias = inp["attn_bias"]

    bs = float(np.sum(attn_bias.astype(np.float64)))
    cbs = _host_cb(np.asarray(frac, np.float32), bs)

    nc = _build_program()

    def _tile_x(x):
        # [T, D] -> x^T [D, T] -> [kc*2+th, 128, 1024] contiguous chunks
        xt = x.T.reshape(KCH, P, 2, 1024).transpose(0, 2, 1, 3)
        return np.ascontiguousarray(xt.reshape(2 * KCH, P, 1024)
                                    ).astype(ml_dtypes.bfloat16)

    def _tile_w(w):
        # W[sl] [256, D] -> W.T [D, 256] -> [p, kc, 256]
        wt = w.T.reshape(KCH, P, C_LOC).transpose(1, 0, 2)
        return np.ascontiguousarray(wt).astype(ml_dtypes.bfloat16)

    in_maps = []
    for c in range(N_CORES):
        b, g = c // H_LOC, c % H_LOC
        sl = slice(g * C_LOC, (g + 1) * C_LOC)
        wo_t = Wo[:, sl].T.reshape(2, P, D).transpose(1, 0, 2)
        in_maps.append({
            "xq": _tile_x(query[b]),
            "xk": _tile_x(key[b]),
            "xv": _tile_x(value[b]),
            "wq": _tile_w(Wq[sl, :]),
            "wk": _tile_w(Wk[sl, :]),
            "wv": _tile_w(Wv[sl, :]),
            "wo": np.ascontiguousarray(wo_t).astype(ml_dtypes.bfloat16),
            "cb": cbs[b],
        })
    return nc, in_maps


def kernel(**inputs) -> np.ndarray:
    nc, in_maps = _prepare(inputs)

    res = run_bass_kernel_spmd(nc, in_maps, list(range(N_CORES)))
    global _last_results
    _last_results = res

    out = np.zeros((B, T, D), dtype=np.float32)
    for c in range(N_CORES):
        out[c // H_LOC] += np.asarray(res.results[c]["out"]).astype(np.float32)
    out += np.asarray(inputs["bo"], dtype=np.float32)[None, None, :]
    return out

